# revision 19
# baseline (speedup 1.0000x reference)
"""ChamferLoss Trainium2 kernel.

Strategy (per core, data-parallel over batch: 16 batches / 8 cores = 2 each):
  pdist[b,i,j] = ||x_i||^2 + ||y_j||^2 - 2 x_i.y_j   (first 3 channels)
  loss = mean_bj(min_i pdist) + mean_bi(min_j pdist)

m = -pdist comes from a single K=13 bf16 augmented matmul (hi/lo split gives
fp32-class accuracy at bf16 PE speed):
  x-side rows: [xh(3), xh(3), xl(3), -rxh, -rxl, -1, -1]
  y-side rows: [Yh(3), Yl(3), Yh(3),  1,    1,  Ryh, Ryl],  Y = 2y, Ry=||y||^2

Main loop is a 3-engine pipeline over row-tiles (x-tile [13,128] x all y):
  PE:  2x4 matmuls fill two PSUM halves [128,2048] f32 (double-buffered)
  ACT: evacuates+casts PSUM f32 -> SBUF bf16 sb[128,4096] (ACTIVATE Copy)
  DVE: colmax via tensor_tensor max into bf16 colacc (2x mode), rowmax via
       a pairwise-max TT fold tree 4096->2048->1024->512 (TT bf16 SBUF is
       the ONLY 2x-capable max op; tensor_scalar/reduce/pool are all 1x)
       + one small reduce per tile pair.
DVE is the saturated engine (~355us busy); ACT ~76%, PE ~70% (HAM-cold,
which doesn't matter off the critical path). GPSIMD can only memset here
(walrus rejects Pool-engine tensor ops).

Prep packs 4 transpose-q's per [128,128] PE transpose (aug channels padded
13->32 so output partitions stay 32-aligned) and evacuates on ScalarE.
Finals: colacc gets PE-transposed (bf16) and max/add-reduced; per-tile
rowmax partials summed. Output is per-partition partial sums
[128, 2*b_loc]; the host does the final 128-way gather-sum.

build_nc(reps=R, hw_loop=True) wraps the whole body in a
tc.For_i(staggered_reset=True) hardware loop: the NEFF size is then
independent of R, so a timing harness can measure marginal device time per
repetition without NEFF-size-dependent host/dispatch overhead polluting
the difference (that overhead — ~0.28us per NEFF byte per execution on the
axon path — is what made the previous python-unrolled-reps baseline report
50ms for an 871us kernel). staggered_reset overlaps loop iterations
(saves ~74us/iter vs the all-engine-barrier back-edge).
"""

from contextlib import ExitStack

import numpy as np

import concourse.bass as bass
import concourse.bacc as bacc
import concourse.tile as tile
from concourse import bass_isa, mybir
from concourse.bass_utils import run_bass_kernel_spmd
from concourse.masks import make_identity

F32 = mybir.dt.float32
BF16 = mybir.dt.bfloat16
AX = mybir.AxisListType
OP = mybir.AluOpType
ACTF = mybir.ActivationFunctionType

NEG_BIG = -3.0e38

B_FULL = 16
N_FULL = 4096
C_FULL = 6
N_CORES = 8


def build_nc(b_loc=2, n=4096, c_in=6, num_devices=8, reps=1, hw_loop=False,
             staggered_reset=True):
    """Per-core program. Inputs x,y: [b_loc, n, c_in] f32; output "partial"
    [128, 2*b_loc] f32 per-partition partial sums of rowmax/colmax of -pdist."""
    NP = 128
    NQ = n // NP                  # row-tiles per batch (32)
    NH = 2                        # half-tiles per row-tile
    HW = n // NH                  # half-tile width (2048)

    nc = bacc.Bacc(
        "TRN2",
        target_bir_lowering=False,
        debug=False,
        enable_asserts=False,
        num_devices=num_devices,
    )

    x_d = nc.declare_dram_parameter("x", [b_loc, n, c_in], F32, isOutput=False).ap()
    y_d = nc.declare_dram_parameter("y", [b_loc, n, c_in], F32, isOutput=False).ap()
    out_d = nc.declare_dram_parameter(
        "partial", [NP, 2 * b_loc], F32, isOutput=True
    ).ap()

    with tile.TileContext(nc) as tc, ExitStack() as ctx:
        prep = ctx.enter_context(tc.tile_pool(name="prep", bufs=2))
        singles = ctx.enter_context(tc.tile_pool(name="singles", bufs=1))
        psum_pool = ctx.enter_context(tc.tile_pool(name="psum", bufs=2, space="PSUM"))
        evac = ctx.enter_context(tc.tile_pool(name="evac", bufs=3))
        smalls = ctx.enter_context(tc.tile_pool(name="smalls", bufs=2))

        ident = singles.tile([NP, NP], BF16, tag="ident", name="ident")
        make_identity(nc, ident)

        def emit_body():
            chx = [singles.tile([13, n], BF16, tag=f"chx{b}", name=f"chx{b}")
                   for b in range(b_loc)]
            chy = [singles.tile([13, n], BF16, tag=f"chy{b}", name=f"chy{b}")
                   for b in range(b_loc)]

            # ---- prep: aug point-major (DVE only), PE-transpose, DVE evac
            for b in range(b_loc):
                for side in ("x", "y"):
                    src = x_d if side == "x" else y_d
                    xin = prep.tile([NP, NQ, c_in], F32, tag="xin")
                    nc.sync.dma_start(
                        out=xin, in_=src[b].rearrange("(p q) c -> p q c", p=NP)
                    )
                    # channel dim padded 13 -> 32 so 4 q's pack into one
                    # [128,128] transpose with 32-aligned output partitions
                    aug = prep.tile([NP, NQ, 32], BF16, tag="aug")
                    sq = prep.tile([NP, NQ, 3], F32, tag="sq")
                    rt = prep.tile([NP, NQ, 1], F32, tag="rt")
                    ch = xin[:, :, 0:3]
                    nc.gpsimd.memset(aug[:, :, 13:32], 0.0)
                    nc.vector.tensor_mul(sq, ch, ch)
                    nc.vector.tensor_reduce(rt, sq, axis=AX.X, op=OP.add)
                    if side == "x":
                        # [xh xh xl | -rxh -rxl | -1 -1]
                        nc.vector.tensor_copy(aug[:, :, 0:3], ch)
                        nc.vector.tensor_copy(aug[:, :, 3:6], aug[:, :, 0:3])
                        nc.vector.tensor_sub(aug[:, :, 6:9], ch, aug[:, :, 0:3])
                        nc.vector.tensor_scalar_mul(aug[:, :, 9:10], rt, -1.0)
                        nc.vector.scalar_tensor_tensor(
                            aug[:, :, 10:11], rt, -1.0, aug[:, :, 9:10],
                            OP.mult, OP.subtract,
                        )
                        nc.vector.memset(aug[:, :, 11:13], -1.0)
                    else:
                        # [Yh Yl Yh | 1 1 | ryh ryl],  Y = 2y
                        nc.vector.tensor_scalar_mul(aug[:, :, 0:3], ch, 2.0)
                        nc.vector.scalar_tensor_tensor(
                            aug[:, :, 3:6], ch, 2.0, aug[:, :, 0:3],
                            OP.mult, OP.subtract,
                        )
                        nc.vector.tensor_copy(aug[:, :, 6:9], aug[:, :, 0:3])
                        nc.vector.memset(aug[:, :, 9:11], 1.0)
                        nc.vector.tensor_copy(aug[:, :, 11:12], rt)
                        nc.vector.tensor_sub(aug[:, :, 12:13], rt, aug[:, :, 11:12])

                    # one PSUM fill: transposes packed 4 q's per instruction
                    # ([128, 128] -> [128, 128]), then strided DVE evacuation
                    PK = 4  # q's packed per transpose
                    NG = NQ // PK
                    pt = psum_pool.tile([NP, NG * NP], BF16, tag="ps")
                    for g in range(NG):
                        nc.tensor.transpose(
                            pt[:, g * NP : (g + 1) * NP],
                            aug[:, g * PK : (g + 1) * PK, :].rearrange(
                                "p q c -> p (q c)"
                            ),
                            ident,
                        )
                    dst = chx[b] if side == "x" else chy[b]
                    # pt row 32*dq+c, col g*128+p  ->  chx[c, (g*PK+dq)*128+p]
                    # evacuation on ScalarE: DVE is the kernel's critical
                    # engine, ACT has slack
                    for dq in range(PK):
                        nc.scalar.activation(
                            dst.rearrange("c (g w p) -> c g w p", w=PK, p=NP)[
                                :, :, dq, :
                            ],
                            pt[32 * dq : 32 * dq + 13, :].rearrange(
                                "c (g p) -> c g p", p=NP
                            ),
                            ACTF.Copy,
                        )

            # ---- accumulators ----
            colacc = [singles.tile([NP, n], BF16, tag=f"colacc{b}",
                                   name=f"colacc{b}") for b in range(b_loc)]
            for b in range(b_loc):
                nc.gpsimd.memset(colacc[b], NEG_BIG)
            rowtile = [singles.tile([NP, NQ], F32, tag=f"rowtile{b}",
                                    name=f"rowtile{b}") for b in range(b_loc)]

            # ---- main: PE fill -> ACT evac/cast -> DVE colmax TT + rowmax
            # fold-tree.  All DVE reduce ops are 1x mode, but TENSOR_TENSOR
            # max on bf16 SBUF runs 2x, so the per-tile rowmax is computed as
            # a pairwise-max fold (4096->2048->1024->512) followed by one
            # small 1x cache-reduce.  sb spans the whole row-tile so every
            # DVE/ACT op is as wide as possible (per-op overhead amortized).
            for b in range(b_loc):
                wpair = None
                for r in range(NQ):
                    lhsT = chx[b][:, r * NP : (r + 1) * NP]
                    sb = evac.tile([NP, n], BF16, tag="sb")
                    for h in range(NH):
                        ps = psum_pool.tile([NP, HW], F32, tag="ps")
                        for s in range(HW // 512):
                            c0 = h * HW + s * 512
                            nc.tensor.matmul(
                                ps[:, s * 512 : (s + 1) * 512],
                                lhsT=lhsT,
                                rhs=chy[b][:, c0 : c0 + 512],
                                start=True,
                                stop=True,
                            )
                        nc.scalar.activation(
                            sb[:, h * HW : (h + 1) * HW], ps, ACTF.Copy
                        )
                    nc.vector.tensor_tensor(colacc[b], colacc[b], sb, op=OP.max)
                    u = smalls.tile([NP, n // 2], BF16, tag="u")
                    nc.vector.tensor_tensor(
                        u, sb[:, 0 : n // 2], sb[:, n // 2 : n], op=OP.max
                    )
                    v = smalls.tile([NP, n // 4], BF16, tag="v")
                    nc.vector.tensor_tensor(
                        v, u[:, 0 : n // 4], u[:, n // 4 : n // 2], op=OP.max
                    )
                    if r % 2 == 0:
                        wpair = smalls.tile([NP, 2, n // 8], BF16, tag="wpair")
                    nc.vector.tensor_tensor(
                        wpair[:, r % 2, :],
                        v[:, 0 : n // 8], v[:, n // 8 : n // 4], op=OP.max
                    )
                    if r % 2 == 1:
                        nc.vector.tensor_reduce(
                            rowtile[b][:, r - 1 : r + 1], wpair,
                            axis=AX.X, op=OP.max,
                        )

            # ---- finals ----
            sums = singles.tile([NP, 2 * b_loc], F32, tag="sums", name="sums")
            for b in range(b_loc):
                # row side: sum the per-tile rowmax partials
                nc.vector.tensor_reduce(sums[:, b : b + 1], rowtile[b],
                                        axis=AX.X, op=OP.add)
                # col side: transpose colacc (bf16), rowmax-reduce, sum
                pt2 = psum_pool.tile([NP, n], BF16, tag="ps")
                for t in range(NQ):
                    nc.tensor.transpose(
                        pt2[:, t * NP : (t + 1) * NP],
                        colacc[b][:, t * NP : (t + 1) * NP],
                        ident,
                    )
                cmax = smalls.tile([NP, NQ], F32, tag="cmax")
                nc.vector.tensor_reduce(
                    cmax, pt2.rearrange("p (t v) -> p t v", t=NQ),
                    axis=AX.X, op=OP.max,
                )
                nc.vector.tensor_reduce(sums[:, b_loc + b : b_loc + b + 1], cmax,
                                        axis=AX.X, op=OP.add)
            nc.sync.dma_start(out=out_d, in_=sums)

        if hw_loop:
            with tc.For_i(0, reps, 1, staggered_reset=staggered_reset):
                emit_body()
        else:
            for _ in range(reps):
                emit_body()

    nc.compile()
    return nc


_CACHE = {}


def _get_nc():
    if "nc" not in _CACHE:
        _CACHE["nc"] = build_nc(
            b_loc=B_FULL // N_CORES, n=N_FULL, c_in=C_FULL, num_devices=N_CORES
        )
    return _CACHE["nc"]


def kernel(x: np.ndarray, y: np.ndarray) -> np.ndarray:
    x = np.ascontiguousarray(np.asarray(x, dtype=np.float32))
    y = np.ascontiguousarray(np.asarray(y, dtype=np.float32))
    assert x.shape == (B_FULL, N_FULL, C_FULL), x.shape
    nc = _get_nc()
    bl = B_FULL // N_CORES
    in_maps = [
        {
            "x": np.ascontiguousarray(x[i * bl : (i + 1) * bl]),
            "y": np.ascontiguousarray(y[i * bl : (i + 1) * bl]),
        }
        for i in range(N_CORES)
    ]
    res = run_bass_kernel_spmd(nc, in_maps, list(range(N_CORES)))
    total = sum(float(r["partial"].astype(np.float64).sum()) for r in res.results)
    loss = -total / float(B_FULL * N_FULL)
    return np.float32(loss)


# revision 22
# speedup vs baseline: 1.0524x; 1.0524x over previous
"""ChamferLoss Trainium2 kernel.

Strategy (per core, data-parallel over batch: 16 batches / 8 cores = 2 each):
  pdist[b,i,j] = ||x_i||^2 + ||y_j||^2 - 2 x_i.y_j   (first 3 channels)
  loss = mean_bj(min_i pdist) + mean_bi(min_j pdist)

m = -pdist comes from a single K=13 bf16 augmented matmul (hi/lo split gives
fp32-class accuracy at bf16 PE speed):
  x-side rows: [xh(3), xh(3), xl(3), -rxh, -rxl, -1, -1]
  y-side rows: [Yh(3), Yl(3), Yh(3),  1,    1,  Ryh, Ryl],  Y = 2y, Ry=||y||^2

Main loop is a 3-engine pipeline over row-tiles (x-tile [13,128] x all y):
  PE:  2x4 matmuls fill two PSUM halves [128,2048] f32 (double-buffered)
  ACT: evacuates+casts PSUM f32 -> SBUF bf16 sb[128,4096] (ACTIVATE Copy)
  DVE: colmax via tensor_tensor max into bf16 colacc (2x mode), rowmax via
       a pairwise-max TT fold tree 4096->2048->1024->512 (TT bf16 SBUF is
       the ONLY 2x-capable max op; tensor_scalar/reduce/pool are all 1x)
       + one small reduce per tile pair.
DVE is the saturated engine (~355us busy); ACT ~76%, PE ~70% (HAM-cold,
which doesn't matter off the critical path). GPSIMD can only memset here
(walrus rejects Pool-engine tensor ops).

Prep packs 4 transpose-q's per [128,128] PE transpose (aug channels padded
13->32 so output partitions stay 32-aligned) and evacuates on ScalarE.
Finals: colacc gets PE-transposed (bf16) and max/add-reduced; per-tile
rowmax partials summed. Output is per-partition partial sums
[128, 2*b_loc]; the host does the final 128-way gather-sum.

build_nc(reps=R, hw_loop=True) wraps the whole body in a
tc.For_i(staggered_reset=True) hardware loop: the NEFF size is then
independent of R, so a timing harness can measure marginal device time per
repetition without NEFF-size-dependent host/dispatch overhead polluting
the difference (that overhead — ~0.28us per NEFF byte per execution on the
axon path — is what made the previous python-unrolled-reps baseline report
50ms for an 871us kernel). staggered_reset overlaps loop iterations
(saves ~74us/iter vs the all-engine-barrier back-edge).
"""

from contextlib import ExitStack

import numpy as np

import concourse.bass as bass
import concourse.bacc as bacc
import concourse.tile as tile
from concourse import bass_isa, mybir
from concourse.bass_utils import run_bass_kernel_spmd
from concourse.masks import make_identity

F32 = mybir.dt.float32
BF16 = mybir.dt.bfloat16
AX = mybir.AxisListType
OP = mybir.AluOpType
ACTF = mybir.ActivationFunctionType

NEG_BIG = -3.0e38

B_FULL = 16
N_FULL = 4096
C_FULL = 6
N_CORES = 8


def build_nc(b_loc=2, n=4096, c_in=6, num_devices=8, reps=1, hw_loop=False,
             staggered_reset=True):
    """Per-core program. Inputs x,y: [b_loc, n, c_in] f32; output "partial"
    [128, 2*b_loc] f32 per-partition partial sums of rowmax/colmax of -pdist."""
    NP = 128
    NQ = n // NP                  # row-tiles per batch (32)
    NH = 2                        # half-tiles per row-tile
    HW = n // NH                  # half-tile width (2048)

    nc = bacc.Bacc(
        "TRN2",
        target_bir_lowering=False,
        debug=False,
        enable_asserts=False,
        num_devices=num_devices,
    )

    x_d = nc.declare_dram_parameter("x", [b_loc, n, c_in], F32, isOutput=False).ap()
    y_d = nc.declare_dram_parameter("y", [b_loc, n, c_in], F32, isOutput=False).ap()
    out_d = nc.declare_dram_parameter(
        "partial", [NP, 2 * b_loc], F32, isOutput=True
    ).ap()

    with tile.TileContext(nc) as tc, ExitStack() as ctx:
        prep = ctx.enter_context(tc.tile_pool(name="prep", bufs=2))
        singles = ctx.enter_context(tc.tile_pool(name="singles", bufs=1))
        psum_pool = ctx.enter_context(tc.tile_pool(name="psum", bufs=2, space="PSUM"))
        evac = ctx.enter_context(tc.tile_pool(name="evac", bufs=3))
        smalls = ctx.enter_context(tc.tile_pool(name="smalls", bufs=2))

        ident = singles.tile([NP, NP], BF16, tag="ident", name="ident")
        make_identity(nc, ident)

        def emit_body():
            chx = [singles.tile([13, n], BF16, tag=f"chx{b}", name=f"chx{b}")
                   for b in range(b_loc)]
            chy = [singles.tile([13, n], BF16, tag=f"chy{b}", name=f"chy{b}")
                   for b in range(b_loc)]

            # ---- prep: aug point-major (DVE only), PE-transpose, DVE evac
            for b in range(b_loc):
                for side in ("x", "y"):
                    src = x_d if side == "x" else y_d
                    xin = prep.tile([NP, NQ, c_in], F32, tag="xin")
                    nc.sync.dma_start(
                        out=xin, in_=src[b].rearrange("(p q) c -> p q c", p=NP)
                    )
                    # channel dim padded 13 -> 32 so 4 q's pack into one
                    # [128,128] transpose with 32-aligned output partitions
                    aug = prep.tile([NP, NQ, 32], BF16, tag="aug")
                    sq = prep.tile([NP, NQ, 3], F32, tag="sq")
                    rt = prep.tile([NP, NQ, 1], F32, tag="rt")
                    ch = xin[:, :, 0:3]
                    nc.gpsimd.memset(aug[:, :, 13:32], 0.0)
                    nc.scalar.square(sq, ch)
                    nc.vector.tensor_reduce(rt, sq, axis=AX.X, op=OP.add)
                    if side == "x":
                        # [xh xh xl | -rxh -rxl | -1 -1]
                        nc.scalar.activation(aug[:, :, 0:3], ch, ACTF.Copy)
                        nc.scalar.activation(aug[:, :, 3:6], ch, ACTF.Copy)
                        nc.vector.tensor_sub(aug[:, :, 6:9], ch, aug[:, :, 0:3])
                        nc.vector.tensor_scalar_mul(aug[:, :, 9:10], rt, -1.0)
                        nc.vector.scalar_tensor_tensor(
                            aug[:, :, 10:11], rt, -1.0, aug[:, :, 9:10],
                            OP.mult, OP.subtract,
                        )
                        nc.gpsimd.memset(aug[:, :, 11:13], -1.0)
                    else:
                        # [Yh Yl Yh | 1 1 | ryh ryl],  Y = 2y
                        nc.scalar.mul(aug[:, :, 0:3], ch, 2.0)
                        nc.vector.scalar_tensor_tensor(
                            aug[:, :, 3:6], ch, 2.0, aug[:, :, 0:3],
                            OP.mult, OP.subtract,
                        )
                        nc.scalar.mul(aug[:, :, 6:9], ch, 2.0)
                        nc.gpsimd.memset(aug[:, :, 9:11], 1.0)
                        nc.scalar.activation(aug[:, :, 11:12], rt, ACTF.Copy)
                        nc.vector.tensor_sub(aug[:, :, 12:13], rt, aug[:, :, 11:12])

                    # one PSUM fill: transposes packed 4 q's per instruction
                    # ([128, 128] -> [128, 128]), then strided DVE evacuation
                    PK = 4  # q's packed per transpose
                    NG = NQ // PK
                    pt = psum_pool.tile([NP, NG * NP], BF16, tag="ps")
                    for g in range(NG):
                        nc.tensor.transpose(
                            pt[:, g * NP : (g + 1) * NP],
                            aug[:, g * PK : (g + 1) * PK, :].rearrange(
                                "p q c -> p (q c)"
                            ),
                            ident,
                        )
                    dst = chx[b] if side == "x" else chy[b]
                    # pt row 32*dq+c, col g*128+p  ->  chx[c, (g*PK+dq)*128+p]
                    # evacuation on ScalarE: DVE is the kernel's critical
                    # engine, ACT has slack
                    for dq in range(PK):
                        nc.scalar.activation(
                            dst.rearrange("c (g w p) -> c g w p", w=PK, p=NP)[
                                :, :, dq, :
                            ],
                            pt[32 * dq : 32 * dq + 13, :].rearrange(
                                "c (g p) -> c g p", p=NP
                            ),
                            ACTF.Copy,
                        )

            # ---- accumulators ----
            colacc = [singles.tile([NP, n], BF16, tag=f"colacc{b}",
                                   name=f"colacc{b}") for b in range(b_loc)]
            for b in range(b_loc):
                nc.gpsimd.memset(colacc[b], NEG_BIG)
            rowtile = [singles.tile([NP, NQ], F32, tag=f"rowtile{b}",
                                    name=f"rowtile{b}") for b in range(b_loc)]

            # ---- main: PE fill -> ACT evac/cast -> DVE colmax TT + rowmax
            # fold-tree.  All DVE reduce ops are 1x mode, but TENSOR_TENSOR
            # max on bf16 SBUF runs 2x, so the per-tile rowmax is computed as
            # a pairwise-max fold (4096->2048->1024->512) followed by one
            # small 1x cache-reduce.  sb spans the whole row-tile so every
            # DVE/ACT op is as wide as possible (per-op overhead amortized).
            for b in range(b_loc):
                wquad = None
                for r in range(NQ):
                    lhsT = chx[b][:, r * NP : (r + 1) * NP]
                    sb = evac.tile([NP, n], BF16, tag="sb")
                    for h in range(NH):
                        ps = psum_pool.tile([NP, HW], F32, tag="ps")
                        for s in range(HW // 512):
                            c0 = h * HW + s * 512
                            nc.tensor.matmul(
                                ps[:, s * 512 : (s + 1) * 512],
                                lhsT=lhsT,
                                rhs=chy[b][:, c0 : c0 + 512],
                                start=True,
                                stop=True,
                            )
                        nc.scalar.activation(
                            sb[:, h * HW : (h + 1) * HW], ps, ACTF.Copy
                        )
                    nc.vector.tensor_tensor(colacc[b], colacc[b], sb, op=OP.max)
                    u = smalls.tile([NP, n // 2], BF16, tag="u")
                    nc.vector.tensor_tensor(
                        u, sb[:, 0 : n // 2], sb[:, n // 2 : n], op=OP.max
                    )
                    v = smalls.tile([NP, n // 4], BF16, tag="v")
                    nc.vector.tensor_tensor(
                        v, u[:, 0 : n // 4], u[:, n // 4 : n // 2], op=OP.max
                    )
                    w = smalls.tile([NP, n // 8], BF16, tag="w")
                    nc.vector.tensor_tensor(
                        w, v[:, 0 : n // 8], v[:, n // 8 : n // 4], op=OP.max
                    )
                    if r % 4 == 0:
                        wquad = smalls.tile([NP, 4, n // 16], BF16, tag="wquad")
                    nc.vector.tensor_tensor(
                        wquad[:, r % 4, :],
                        w[:, 0 : n // 16], w[:, n // 16 : n // 8], op=OP.max
                    )
                    if r % 4 == 3:
                        nc.vector.tensor_reduce(
                            rowtile[b][:, r - 3 : r + 1], wquad,
                            axis=AX.X, op=OP.max,
                        )

            # ---- finals ----
            sums = singles.tile([NP, 2 * b_loc], F32, tag="sums", name="sums")
            for b in range(b_loc):
                # row side: sum the per-tile rowmax partials
                nc.vector.tensor_reduce(sums[:, b : b + 1], rowtile[b],
                                        axis=AX.X, op=OP.add)
                # col side: transpose colacc (bf16), rowmax-reduce, sum
                pt2 = psum_pool.tile([NP, n], BF16, tag="ps")
                for t in range(NQ):
                    nc.tensor.transpose(
                        pt2[:, t * NP : (t + 1) * NP],
                        colacc[b][:, t * NP : (t + 1) * NP],
                        ident,
                    )
                cmax = smalls.tile([NP, NQ], F32, tag="cmax")
                nc.vector.tensor_reduce(
                    cmax, pt2.rearrange("p (t v) -> p t v", t=NQ),
                    axis=AX.X, op=OP.max,
                )
                nc.vector.tensor_reduce(sums[:, b_loc + b : b_loc + b + 1], cmax,
                                        axis=AX.X, op=OP.add)
            nc.sync.dma_start(out=out_d, in_=sums)

        if hw_loop:
            with tc.For_i(0, reps, 1, staggered_reset=staggered_reset):
                emit_body()
        else:
            for _ in range(reps):
                emit_body()

    nc.compile()
    return nc


_CACHE = {}


def _get_nc():
    if "nc" not in _CACHE:
        _CACHE["nc"] = build_nc(
            b_loc=B_FULL // N_CORES, n=N_FULL, c_in=C_FULL, num_devices=N_CORES
        )
    return _CACHE["nc"]


def kernel(x: np.ndarray, y: np.ndarray) -> np.ndarray:
    x = np.ascontiguousarray(np.asarray(x, dtype=np.float32))
    y = np.ascontiguousarray(np.asarray(y, dtype=np.float32))
    assert x.shape == (B_FULL, N_FULL, C_FULL), x.shape
    nc = _get_nc()
    bl = B_FULL // N_CORES
    in_maps = [
        {
            "x": np.ascontiguousarray(x[i * bl : (i + 1) * bl]),
            "y": np.ascontiguousarray(y[i * bl : (i + 1) * bl]),
        }
        for i in range(N_CORES)
    ]
    res = run_bass_kernel_spmd(nc, in_maps, list(range(N_CORES)))
    total = sum(float(r["partial"].astype(np.float64).sum()) for r in res.results)
    loss = -total / float(B_FULL * N_FULL)
    return np.float32(loss)
